# revision 1
# baseline (speedup 1.0000x reference)
"""Multi-head attention (BN-folded QKV + rel-pos bias + GELU + out-proj) on 8 TRN2 cores.

Data-parallel over batch (b=8 -> 1 batch element per core, no collectives).

All BatchNorms are eval-mode affine transforms folded into the projection
weights/biases on the host.  The additive Toeplitz position bias is folded in
multiplicatively after exp:  exp(dots + E) = exp(dots) * exp(E), with exp(E)
shipped as per-partition pre-shifted window tiles so each [j, i] tile of the
attention matrix multiplies a plain strided SBUF view.

Layout choice: dots are computed transposed (dotsT[j, i]) so q/k feed the
TensorEngine exactly as the projections produce them, v is produced already
transposed ([j, dv]) by swapping matmul operands, and a ones-column in v
makes the softmax denominators fall out of the attn@v matmul for free.
"""

import numpy as np
import ml_dtypes

HEADS, DK, DV = 8, 32, 64
DIM, N, DIM_OUT = 256, 1024, 256
IDK, IDV = DK * HEADS, DV * HEADS  # 256, 512
SCALE = DK ** -0.5
EPS = 1e-5
B = 8
WIN = 1920  # window tile width: covers i - 128*jc in [-896, 1023]

BF16 = ml_dtypes.bfloat16


def _prep_host(x, Wq, Wk, Wv, Wo, bo, pos_emb,
               q_gamma, q_beta, q_mean, q_var,
               k_gamma, k_beta, k_mean, k_var,
               v_gamma, v_beta, v_mean, v_var,
               o_gamma, o_beta, o_mean, o_var):
    f32 = np.float32
    inv_q = (q_gamma / np.sqrt(q_var + EPS)).astype(f32)
    inv_k = (k_gamma / np.sqrt(k_var + EPS)).astype(f32)
    inv_v = (v_gamma / np.sqrt(v_var + EPS)).astype(f32)
    inv_o = (o_gamma / np.sqrt(o_var + EPS)).astype(f32)

    # q also absorbs the attention scale
    Wq_eff = (Wq * inv_q[:, None]) * SCALE
    bq = ((q_beta - q_mean * inv_q) * SCALE).astype(f32)
    Wk_eff = Wk * inv_k[:, None]
    bk = (k_beta - k_mean * inv_k).astype(f32)
    Wv_eff = Wv * inv_v[:, None]
    bv = (v_beta - v_mean * inv_v).astype(f32)
    Wo_eff = Wo * inv_o[:, None]
    bo_eff = ((bo - o_mean) * inv_o + o_beta).astype(f32)

    # lhsT layouts, pre-chunked to the exact SBUF tile shapes
    def chunk_T(w, kchunks):  # [O, C] -> [128, kchunks, O]  (WT[c, o] tiled)
        wT = np.ascontiguousarray(w.T.astype(f32))  # [C, O]
        c, o = wT.shape
        assert c == kchunks * 128
        return np.ascontiguousarray(
            wT.reshape(kchunks, 128, o).transpose(1, 0, 2)).astype(BF16)

    wqs = chunk_T(Wq_eff, 2)            # [128, 2, 256]
    wks = chunk_T(Wk_eff, 2)            # [128, 2, 256]
    wvs = chunk_T(Wv_eff, 2)            # [128, 2, 512]
    wos = chunk_T(Wo_eff, 4)            # [128, 4, 256]

    def chunk_bias(b, chunks):  # [C] -> [128, chunks]
        return np.ascontiguousarray(
            b.reshape(chunks, 128).T).astype(f32)

    bqs = chunk_bias(bq, 2)             # [128, 2]
    bks = chunk_bias(bk, 2)             # [128, 2]
    bos = chunk_bias(bo_eff, 2)         # [128, 2]
    bvrow = bv.reshape(1, IDV).astype(BF16)  # [1, 512]  (h-major: c = h*64+dv)

    # exp-window tiles: win[p, h, u] = exp(pos_emb[|u - p + 127 - 1023 + 1023|]...)
    # indexed so that for j = 128*jc + p, attn[p, i] needs
    # expE[1023 + i - j] = win[p, h, (896 - 128*jc) + i]
    E = (np.asarray(pos_emb, dtype=np.float64) / SCALE)  # [N, HEADS]
    d = np.abs(np.arange(2047) - 1023)
    expE = np.exp(E[d, :])  # [2047, HEADS] float64
    idx = np.arange(WIN)[None, :] - np.arange(128)[:, None] + 127  # [128, WIN]
    win = expE[idx, :].transpose(0, 2, 1)  # [128, HEADS, WIN]
    win = np.ascontiguousarray(win).astype(BF16)

    shared = dict(wqs=wqs, wks=wks, wvs=wvs, wos=wos,
                  bqs=bqs, bks=bks, bos=bos, bvrow=bvrow, win=win)
    return shared


def _x_shard(x, i):
    # device consumes x as bf16 [128, 2, n] (channel chunks c = a*128 + p)
    xi = np.asarray(x[i], dtype=np.float32).reshape(2, 128, N).transpose(1, 0, 2)
    return np.ascontiguousarray(xi).astype(BF16)


def _build_nc():
    import concourse.bass as bass
    import concourse.mybir as mybir
    import concourse.tile as tile
    from concourse import bacc

    f32 = mybir.dt.float32
    bf16 = mybir.dt.bfloat16

    nc = bacc.Bacc(None, target_bir_lowering=False)

    x_ext = nc.declare_dram_parameter("x", [128, 2, N], bf16, isOutput=False)
    wqs_ext = nc.declare_dram_parameter("wqs", [128, 2, IDK], bf16, isOutput=False)
    wks_ext = nc.declare_dram_parameter("wks", [128, 2, IDK], bf16, isOutput=False)
    wvs_ext = nc.declare_dram_parameter("wvs", [128, 2, IDV], bf16, isOutput=False)
    wos_ext = nc.declare_dram_parameter("wos", [128, 4, DIM_OUT], bf16, isOutput=False)
    bqs_ext = nc.declare_dram_parameter("bqs", [128, 2], f32, isOutput=False)
    bks_ext = nc.declare_dram_parameter("bks", [128, 2], f32, isOutput=False)
    bos_ext = nc.declare_dram_parameter("bos", [128, 2], f32, isOutput=False)
    bvrow_ext = nc.declare_dram_parameter("bvrow", [1, IDV], bf16, isOutput=False)
    win_ext = nc.declare_dram_parameter("win", [128, HEADS, WIN], bf16, isOutput=False)
    out_ext = nc.declare_dram_parameter("out", [DIM_OUT, N], f32, isOutput=True)

    Exp = mybir.ActivationFunctionType.Exp
    Gelu = mybir.ActivationFunctionType.Gelu

    with tile.TileContext(nc) as tc:
        with (
            tc.tile_pool(name="consts", bufs=1) as consts,
            tc.tile_pool(name="scratch", bufs=2) as scratch,
            tc.tile_pool(name="attnp", bufs=12) as attnp,
            tc.tile_pool(name="normp", bufs=6) as normp,
            tc.tile_pool(name="psum", bufs=2, space="PSUM") as psum,
        ):
            # ---- load constants (x + q/k weights first: they gate the first matmuls) ----
            xb = consts.tile([128, 2, N], bf16)
            nc.sync.dma_start(out=xb[:, 0, :], in_=x_ext[:, 0, :])
            nc.sync.dma_start(out=xb[:, 1, :], in_=x_ext[:, 1, :])
            wq = consts.tile([128, 2, IDK], bf16)
            nc.sync.dma_start(out=wq, in_=wqs_ext[:])
            wk = consts.tile([128, 2, IDK], bf16)
            nc.sync.dma_start(out=wk, in_=wks_ext[:])
            bqs = consts.tile([128, 2], f32)
            nc.sync.dma_start(out=bqs, in_=bqs_ext[:])
            bks = consts.tile([128, 2], f32)
            nc.sync.dma_start(out=bks, in_=bks_ext[:])
            wv = consts.tile([128, 2, IDV], bf16)
            nc.sync.dma_start(out=wv, in_=wvs_ext[:])
            bvr = consts.tile([1, IDV], bf16)
            nc.sync.dma_start(out=bvr, in_=bvrow_ext[:])
            win = consts.tile([128, HEADS, WIN], bf16)
            nc.sync.dma_start(out=win, in_=win_ext[:])
            wo = consts.tile([128, 4, DIM_OUT], bf16)
            nc.sync.dma_start(out=wo, in_=wos_ext[:])
            bos = consts.tile([128, 2], f32)
            nc.sync.dma_start(out=bos, in_=bos_ext[:])
            ones1 = consts.tile([1, 128], bf16)
            nc.vector.memset(ones1, 1.0)
            # dummy exp: walrus inserts the exp table load before ACT's first
            # Exp -- placing one here pulls the ~2.7us load into the DMA wait
            # instead of the first attention pair's critical path
            warm = scratch.tile([1, 8], f32, tag="warm")
            nc.vector.memset(warm, 1.0)
            nc.scalar.activation(warm, warm, Exp)

            # ---- persistent intermediates ----
            q_sb = consts.tile([128, 2, N], bf16)   # [ (h,d) chunks, i ]
            k_sb = consts.tile([128, 2, N], bf16)   # [ (h,d) chunks, j ]
            # v columns 64:128 per head, columns 0:32 all-ones (sums come out
            # 32-replicated at base partition 0 -- safe for the custom-DVE
            # reciprocal), columns 32:64 zero (dark PE cells), out_u on
            # partitions 1:65.  M=65 keeps half the PE array dark vs M=128 --
            # same cycles, less power for the activity throttler to punish.
            v_aug = consts.tile([128, 8, HEADS, 128], bf16)  # [j-part, jc, h, one|0|dv]
            g_sb = consts.tile([128, 4, N], bf16)   # gelu input/output [(h,dv) chunks, i]

            # ---- q/k projections (mc=0 first: heads 0-3 gate the first
            # attention pair; evacuation on the otherwise-idle ACT engine) ----
            Identity = mybir.ActivationFunctionType.Identity
            for mc in range(2):
                for (w_t, b_t, dst) in ((wq, bqs, q_sb), (wk, bks, k_sb)):
                    for ic in range(2):
                        ps = psum.tile([128, 512], f32, tag="ops", bufs=4)
                        for kc in range(2):
                            nc.tensor.matmul(
                                ps,
                                lhsT=w_t[:, kc, mc * 128:(mc + 1) * 128],
                                rhs=xb[:, kc, ic * 512:(ic + 1) * 512],
                                start=(kc == 0), stop=(kc == 1))
                        nc.scalar.activation(
                            dst[:, mc, ic * 512:(ic + 1) * 512], ps,
                            Identity, bias=b_t[:, mc:mc + 1])

            # ---- v projection, produced transposed: v_aug[j, (h, dv)] ----
            nc.vector.memset(v_aug[:, :, :, 0:32], 1.0)
            nc.vector.memset(v_aug[:, :, :, 32:DV], 0.0)

            def emit_vproj(jc):
                ps = psum.tile([128, 512], f32, tag="ops", bufs=4,
                               name=f"vps_{jc}")
                for kc in range(2):
                    nc.tensor.matmul(
                        ps,
                        lhsT=xb[:, kc, jc * 128:(jc + 1) * 128],
                        rhs=wv[:, kc, :],
                        start=(kc == 0), stop=False)
                # + bias via ones-row x bvrow (K=1)
                nc.tensor.matmul(ps, lhsT=ones1, rhs=bvr,
                                 start=False, stop=True)
                nc.vector.tensor_copy(
                    v_aug[:, jc, :, DV:128],
                    ps.rearrange("p (h d) -> p h d", h=HEADS))

            # ---- attention: head pairs; the two heads' K=32 dots matmuls share
            # the PE array via distinct 32-row groups (concurrent), two dots
            # tiles -> one exp + one pair-strided window multiply each ----
            pending_norms = []

            def emit_norm_chain(h, ic, ops):
                # g[dv, i] = out_u[dv, i] / sums[i]  (+bv folded into v)
                # reciprocal lands directly in the broadcast tile's first 32
                # partitions; one SB->SB DMA widens to 64 (SDMA ports are
                # independent of the DVE/GpSimd port pair -- no engine stalls)
                bc = normp.tile([DV, 512], f32, tag="bc", name=f"bc_{h}_{ic}")
                nc.vector.reciprocal_approx_fast(bc[0:32, :], ops[0:32, :])
                nc.sync.dma_start(out=bc[32:DV, :], in_=bc[0:32, :])
                nc.vector.tensor_mul(
                    g_sb[(h % 2) * DV:(h % 2) * DV + DV, h // 2,
                         ic * 512:(ic + 1) * 512],
                    ops[DV:128, :], bc)

            for jc in range(8):
                emit_vproj(jc)

            units = [(p, jc) for p in range(4) for jc in range(8)]
            pair_state = {}
            attnv_q = []

            def emit_attnv(p, jc, at):
                st = pair_state[p]
                h0, h1 = 2 * p, 2 * p + 1
                for ic in range(2):
                    nc.tensor.matmul(
                        st[0][ic], lhsT=v_aug[:, jc, h0, :],
                        rhs=at[:, ic, 0, :],
                        start=(jc == 0), stop=(jc == 7))
                    nc.tensor.matmul(
                        st[1][ic], lhsT=v_aug[:, jc, h1, :],
                        rhs=at[:, ic, 1, :],
                        start=(jc == 0), stop=(jc == 7))

            import concourse.bass as bass_mod
            for p, jc in units:
                h0, h1 = 2 * p, 2 * p + 1
                koff0, kch0 = (h0 % 4) * 32, h0 // 4
                koff1, kch1 = (h1 % 4) * 32, h1 // 4
                if jc == 0:
                    pair_state[p] = [[psum.tile([128, 512], f32, tag="ops", bufs=4,
                                                name=f"ops_{h}_{ic}")
                                      for ic in range(2)] for h in (h0, h1)]
                off = 896 - 128 * jc
                # attn layout: [128, (ic, head-half, 512)]
                attn = attnp.tile([128, 2, 2, 512], bf16, tag="attn",
                                  name=f"attn_{p}_{jc}")
                for ic in range(2):
                    dps = psum.tile([128, N], f32, tag="dots", bufs=2,
                                    name=f"dots_{p}_{jc}_{ic}")
                    nc.tensor.matmul(
                        dps[:, 0:512],
                        lhsT=k_sb[koff0:koff0 + 32, kch0, jc * 128:(jc + 1) * 128],
                        rhs=q_sb[koff0:koff0 + 32, kch0, ic * 512:(ic + 1) * 512],
                        start=True, stop=True, tile_position=(koff0, 0))
                    nc.tensor.matmul(
                        dps[:, 512:1024],
                        lhsT=k_sb[koff1:koff1 + 32, kch1, jc * 128:(jc + 1) * 128],
                        rhs=q_sb[koff1:koff1 + 32, kch1, ic * 512:(ic + 1) * 512],
                        start=True, stop=True, tile_position=(koff1, 0))
                    nc.scalar.activation(attn[:, ic, :, :], dps, Exp)
                # one window multiply for the whole (pair, jc) tile: the
                # attn@v consumers lag 2 units, so waiting on both exps here
                # costs no latency, and halving the DVE op count saves the
                # per-instruction overhead
                wv_view = win[:, h0, off:off + 512]
                wv_quad = bass_mod.AP(
                    tensor=wv_view.tensor, offset=wv_view.offset,
                    ap=[list(wv_view.ap[0]), [512, 2], [WIN, 2], [1, 512]])
                nc.vector.tensor_mul(attn, attn, wv_quad)
                attnv_q.append((p, jc, attn))
                # attn@v two units behind: PE always has fresh dots work queued
                if len(attnv_q) > 2:
                    emit_attnv(*attnv_q.pop(0))
                # norms only after the lagging attn@v units of their pair have
                # been emitted (attnv lags 2 units -> safe from jc >= 2)
                if pending_norms and jc >= 2:
                    emit_norm_chain(*pending_norms.pop(0))
                if jc == 7:
                    pending_norms = [(h0, 0, pair_state[p][0][0]),
                                     (h1, 0, pair_state[p][1][0]),
                                     (h0, 1, pair_state[p][0][1]),
                                     (h1, 1, pair_state[p][1][1])]
            while attnv_q:
                emit_attnv(*attnv_q.pop(0))

            # ---- tail: pair 3's norms (DVE) overlap gelu + the kc 0-2 final
            # projection accumulation (heads 0-5 are normalized long ago; all
            # exps are done so the gelu table loads exactly once) ----
            out_r = out_ext[:].rearrange("(a p) n -> p a n", p=128)
            for args in pending_norms:
                emit_norm_chain(*args)
            pending_norms = []
            fps_t = {}
            for ic in range(2):
                g03 = g_sb[:, 0:3, ic * 512:(ic + 1) * 512]
                nc.scalar.activation(g03, g03, Gelu)
                for mc in range(2):
                    fps = psum.tile([128, 512], f32, tag="ops", bufs=4,
                                    name=f"fin_{mc}_{ic}")
                    fps_t[(mc, ic)] = fps
                    for kc in range(3):
                        nc.tensor.matmul(
                            fps,
                            lhsT=wo[:, kc, mc * 128:(mc + 1) * 128],
                            rhs=g_sb[:, kc, ic * 512:(ic + 1) * 512],
                            start=(kc == 0), stop=False)
            for ic in range(2):
                g3 = g_sb[:, 3:4, ic * 512:(ic + 1) * 512]
                nc.scalar.activation(g3, g3, Gelu)
                for mc in range(2):
                    fps = fps_t[(mc, ic)]
                    nc.tensor.matmul(
                        fps,
                        lhsT=wo[:, 3, mc * 128:(mc + 1) * 128],
                        rhs=g_sb[:, 3, ic * 512:(ic + 1) * 512],
                        start=False, stop=True)
                    o_sb = scratch.tile([128, 512], f32, tag="osb",
                                        bufs=4, name=f"osb_{mc}_{ic}")
                    nc.vector.tensor_scalar_add(o_sb, fps, bos[:, mc:mc + 1])
                    nc.sync.dma_start(out=out_r[:, mc, ic * 512:(ic + 1) * 512],
                                      in_=o_sb)

    nc.finalize()
    return nc


_NC_CACHE = None


def kernel(**inputs) -> np.ndarray:
    global _NC_CACHE
    from concourse.bass_utils import run_bass_kernel_spmd

    x = np.asarray(inputs["x"], dtype=np.float32)
    shared = _prep_host(**inputs)

    if _NC_CACHE is None:
        _NC_CACHE = _build_nc()
    nc = _NC_CACHE

    in_maps = [dict(x=_x_shard(x, i), **shared) for i in range(B)]
    res = run_bass_kernel_spmd(nc, in_maps, core_ids=list(range(B)))
    out = np.stack([res.results[i]["out"] for i in range(B)], axis=0)
    return out.astype(np.float32)

